# revision 7
# baseline (speedup 1.0000x reference)
"""Trainium2 Bass kernel for the BiDAF-style attention-flow layer.

S[b,t,j] = H.w_h + U.w_u + (H*w_hu).U + bias
c2q      = softmax_j(S) @ U
q2c      = softmax_t(max_j S) @ H   (broadcast over t)
out      = concat([H, c2q, H*c2q, H*q2c], axis=-1)

Sharding: data-parallel over batch B=64 across 8 NeuronCores (8 batches per
core); W/b replicated; no collectives.

The device computes and stores only the three derived segments
[c2q | H*c2q | H*q2c] in bf16; the H echo segment is assembled on the host
from the (exact f32) input during unsharding, and the bf16 segments are
upcast to f32 on the host. The rel-err budget (2e-2) is ~10x above bf16
rounding noise (~1e-3).

Math notes (exact up to fp rounding):
 - softmax_j(S) is invariant to the per-row sH[t] term, so the c2q path
   uses E = exp(S_core + sU + b) only.
 - q2c logits m[t] = sH[t] + max_j(S_core + sU + b); em = exp(m) is
   computed as exp(sH) * max_j E  (same thing, avoids a second S layout).
 - S is computed TRANSPOSED as stp[j, t] (rows 0:64 = S_core^T + nothing,
   row 64 = sH) via one pair of batched matmuls with batch-constant
   stationary weights [w_hu*U^T | w_h]; sU+b is folded into the exp
   activation bias that produces E^T directly.
"""

import numpy as np

import concourse.bass as bass
import concourse.mybir as mybir
import concourse.tile as tile
from concourse.bass_utils import run_bass_kernel_spmd
from concourse.masks import make_identity

B, T, J, D = 64, 1024, 64, 256
NCORES = 8
BL = B // NCORES  # batches per core
NT = T // 128     # t-tiles per batch
F32 = mybir.dt.float32
BF16 = mybir.dt.bfloat16
AX = mybir.AxisListType.X
AF = mybir.ActivationFunctionType
MUL = mybir.AluOpType.mult
ADD = mybir.AluOpType.add


def split_multi_waits(nc, max_waits=1):
    """Walrus in this container rejects instructions with more than a couple
    of embedded sync waits. Hoist extras into standalone EventSemaphore
    instructions right before the offending instruction."""
    n = 0
    for fn in nc.m.functions:
        for bb in fn.blocks:
            new_insts = []
            for inst in bb.instructions:
                si = getattr(inst, "sync_info", None)
                if si is not None and si.on_wait and len(si.on_wait) > max_waits:
                    waits = list(si.on_wait)
                    for w in waits[:-max_waits]:
                        n += 1
                        ev = mybir.InstEventSemaphore(
                            name=f"I-wsplit-{n}", ins=[], outs=[]
                        )
                        ev.engine = inst.engine
                        ev.sync_info = mybir.SyncInfo(on_wait=[w], on_update=[])
                        new_insts.append(ev)
                    inst.sync_info = mybir.SyncInfo(
                        on_wait=waits[-max_waits:], on_update=list(si.on_update)
                    )
                new_insts.append(inst)
            bb.instructions[:] = new_insts
    return n


def build_nc():
    nc = bass.Bass()
    H = nc.declare_dram_parameter("H", [BL, T, D], F32, isOutput=False)
    U = nc.declare_dram_parameter("U", [BL, J, D], F32, isOutput=False)
    W = nc.declare_dram_parameter("W", [3 * D], F32, isOutput=False)
    b = nc.declare_dram_parameter("b", [1], F32, isOutput=False)
    out = nc.declare_dram_parameter("out", [BL, T, 3 * D], BF16, isOutput=True)

    with tile.TileContext(nc) as tc:
        with (
            tc.tile_pool(name="singles", bufs=1) as singles,
            tc.tile_pool(name="hpool", bufs=2) as hpool,
            tc.tile_pool(name="upool", bufs=2) as upool,
            tc.tile_pool(name="htp", bufs=2) as htpool,
            tc.tile_pool(name="outp", bufs=3) as outp,
            tc.tile_pool(name="batch", bufs=2) as bpool,
            tc.tile_pool(name="small", bufs=4) as small,
            tc.tile_pool(name="ps_a", bufs=2, space="PSUM") as ps_a,
            tc.tile_pool(name="ps_s", bufs=1, space="PSUM") as ps_s,
            tc.tile_pool(name="ps_et", bufs=1, space="PSUM") as ps_et,
            tc.tile_pool(name="ps_cq", bufs=2, space="PSUM") as ps_cq,
            tc.tile_pool(name="ps_qz", bufs=1, space="PSUM") as ps_qz,
        ):
            ident_bf = singles.tile([128, 128], BF16)
            make_identity(nc, ident_bf[:])
            ones_row_bf = singles.tile([1, 128], BF16)
            nc.vector.memset(ones_row_bf[:], 1.0)

            # w_u broadcast over 64 partitions (for the sU reduction)
            w_u_bc = singles.tile([J, D], F32)
            wsl = W[D : 2 * D]
            nc.sync.dma_start(
                out=w_u_bc[:],
                in_=bass.AP(tensor=wsl.tensor, offset=wsl.offset,
                            ap=[[0, J]] + list(wsl.ap)),
            )
            # b broadcast over 64 partitions (initial value of the sU reduce)
            b_col = singles.tile([J, 1], F32)
            bsl = b[0:1]
            nc.sync.dma_start(
                out=b_col[:],
                in_=bass.AP(tensor=bsl.tensor, offset=bsl.offset,
                            ap=[[0, J]] + list(bsl.ap)),
            )
            # w_hu and w_h as [128,2] column blocks
            whu_col = singles.tile([128, 2], F32)
            wh_col = singles.tile([128, 2], F32)
            for k in range(2):
                nc.sync.dma_start(
                    out=whu_col[:, k : k + 1],
                    in_=W[2 * D + 128 * k : 2 * D + 128 * (k + 1)].rearrange(
                        "(p o) -> p o", o=1
                    ),
                )
                nc.sync.dma_start(
                    out=wh_col[:, k : k + 1],
                    in_=W[128 * k : 128 * (k + 1)].rearrange("(p o) -> p o", o=1),
                )

            def load_batch(bi):
                # H cast-loaded f32 -> bf16 during DMA (SWDGE), with a ones
                # column in front of each tile block (for the q2czt matmul).
                hb = hpool.tile([128, NT, D + 1], BF16, tag="hb")
                nc.gpsimd.memset(hb[:, :, 0:1], 1.0)
                nc.gpsimd.dma_start(
                    out=hb[:, :, 1 : D + 1],
                    in_=H[bi].rearrange("(n p) d -> p n d", p=128),
                )
                # U cast-loaded with a trailing ones column (Z via c2q matmul)
                ub = upool.tile([J, D + 1], BF16, tag="ub")
                nc.gpsimd.dma_start(out=ub[:, 0:D], in_=U[bi])
                nc.vector.memset(ub[:, D : D + 1], 1.0)
                return hb, ub

            cur = load_batch(0)

            for bi in range(BL):
                hb, ub = cur
                if bi + 1 < BL:
                    cur = load_batch(bi + 1)

                # ---- per-batch U prep ------------------------------------
                # sU[j] + b  (walrus here rejects tensor_tensor_reduce)
                su_scr = bpool.tile([J, D], F32, tag="suscr")
                su_raw = bpool.tile([J, 1], F32, tag="suraw")
                su_col = bpool.tile([J, 1], F32, tag="sucol")
                nc.vector.tensor_mul(su_scr[:], ub[:, 0:D], w_u_bc[:])
                nc.vector.reduce_sum(su_raw[:], su_scr[:], axis=AX, op=ADD)
                nc.vector.tensor_tensor(
                    out=su_col[:], in0=su_raw[:], in1=b_col[:], op=ADD
                )
                # U^T chunks (bf16), then stationary weights
                # rhs_w[:, c, 0:64] = (w_hu * U^T) chunk c ; [:, c, 64] = w_h
                utp = ps_a.tile([128, 2, J], BF16, tag="ht")
                nc.tensor.transpose(utp[:, 0, :], ub[:, 0:128],
                                    ident_bf[0:J, 0:J])
                nc.tensor.transpose(utp[:, 1, :], ub[:, 128:256],
                                    ident_bf[0:J, 0:J])
                rhs_w = bpool.tile([128, 2, J + 1], BF16, tag="rhsw")
                for c in range(2):
                    nc.vector.tensor_scalar_mul(
                        rhs_w[:, c, 0:J], utp[:, c, :], whu_col[:, c : c + 1]
                    )
                    nc.scalar.copy(rhs_w[:, c, J : J + 1], wh_col[:, c : c + 1])

                # ---- H^T for the whole batch -----------------------------
                ht_all = htpool.tile([128, 2, T], BF16, tag="hta")
                for ti in range(NT):
                    htp = ps_a.tile([128, 2, 128], BF16, tag="ht")
                    nc.tensor.transpose(htp[:, 0, :], hb[:, ti, 1:129],
                                        ident_bf[:])
                    nc.tensor.transpose(htp[:, 1, :], hb[:, ti, 129:257],
                                        ident_bf[:])
                    nc.vector.tensor_copy(
                        ht_all[:, :, 128 * ti : 128 * (ti + 1)], htp[:]
                    )

                # ---- S^T batched: stp_h[j, t] rows 0:64 = S_core^T, 64 = sH
                stps = []
                for h in range(2):
                    stp = ps_s.tile([J + 1, T // 2], F32, tag=f"s{h}")
                    for c in range(2):
                        nc.tensor.matmul(
                            stp[:], rhs_w[:, c, :],
                            ht_all[:, c, 512 * h : 512 * (h + 1)],
                            start=(c == 0), stop=(c == 1),
                        )
                    stps.append(stp)

                # ---- per-tile softmax + c2q ------------------------------
                ot3 = outp.tile([128, NT, 3 * D], BF16, tag="ot")
                q2czt = ps_qz.tile([1, D + 1], F32, tag="qz")
                prev_q2c = None  # lag the q2czt matmul by one tile
                for ti in range(NT):
                    h, q = divmod(ti, 4)
                    scols = slice(128 * q, 128 * (q + 1))
                    stp = stps[h]
                    # E^T = exp(S_core^T + sU + b), sH appended as row 64
                    et_ext = small.tile([J + 1, 128], BF16, tag="etx")
                    nc.scalar.activation(et_ext[0:J, :], stp[0:J, scols],
                                         AF.Exp, bias=su_col[:], scale=1.0)
                    nc.scalar.copy(et_ext[J : J + 1, :], stp[J : J + 1, scols])
                    # transpose back: et_T[t, 0:64] = E, col 64 = sH
                    et_T = ps_et.tile([128, J + 1], BF16, tag="et")
                    nc.tensor.transpose(et_T[:], et_ext[:],
                                        ident_bf[0 : J + 1, 0 : J + 1])
                    # c2q (unnormalized) + Z via the ones column of ub
                    cq = ps_cq.tile([128, D + 1], F32, tag="cq")
                    nc.tensor.matmul(cq[:], et_ext[0:J, :], ub[:],
                                     start=True, stop=True)
                    if prev_q2c is not None:
                        nc.tensor.matmul(**prev_q2c, skip_group_check=True)
                    # em = exp(sH) * max_j E  (= exp(sH + max_j logits))
                    r = small.tile([128, 1], BF16, tag="r")
                    nc.vector.reduce_max(r[:], et_T[:, 0:J], axis=AX,
                                         op=mybir.AluOpType.max)
                    em_a = small.tile([128, 1], BF16, tag="ema")
                    nc.scalar.activation(em_a[:], et_T[:, J : J + 1], AF.Exp)
                    em = small.tile([128, 1], BF16, tag="em")
                    nc.vector.tensor_mul(em[:], em_a[:], r[:])
                    # q2czt += em^T @ [1 | H]  (col 0 = Z_t, 1:257 = em.H)
                    prev_q2c = dict(
                        out=q2czt[0:1, :], lhsT=em[:], rhs=hb[:, ti, :],
                        start=(ti == 0), stop=(ti == NT - 1),
                    )
                    # normalize: seg0 = c2q, seg1 = H * c2q
                    zinv = small.tile([128, 1], F32, tag="zinv")
                    nc.vector.reciprocal(zinv[:], cq[:, D : D + 1])
                    nc.scalar.activation(ot3[:, ti, 0:D], cq[:, 0:D],
                                         AF.Copy, scale=zinv[:])
                    nc.vector.scalar_tensor_tensor(
                        out=ot3[:, ti, D : 2 * D], in0=cq[:, 0:D],
                        scalar=zinv[:], in1=hb[:, ti, 1 : D + 1],
                        op0=MUL, op1=MUL,
                    )
                # issue the lagged last q2czt matmul
                nc.tensor.matmul(**prev_q2c, skip_group_check=True)

                # ---- q2c broadcast + pass 2 ------------------------------
                ztinv = bpool.tile([1, 1], F32, tag="ztinv")
                nc.vector.reciprocal(ztinv[:], q2czt[0:1, 0:1])
                q2c_row = bpool.tile([1, D], BF16, tag="q2crow")
                nc.vector.tensor_scalar_mul(q2c_row[:], q2czt[0:1, 1 : D + 1],
                                            ztinv[:])
                q2cbp = ps_qz.tile([128, D], F32, tag="qz")
                nc.tensor.matmul(q2cbp[:], ones_row_bf[:], q2c_row[:],
                                 start=True, stop=True)
                q2cb = bpool.tile([128, D], BF16, tag="q2cb")
                nc.scalar.copy(q2cb[:], q2cbp[:])

                for ti in range(NT):
                    nc.gpsimd.tensor_mul(
                        ot3[:, ti, 2 * D : 3 * D], hb[:, ti, 1 : D + 1],
                        q2cb[:],
                    )
                # one batched store of all three segments
                nc.scalar.dma_start(
                    out=out[bi].rearrange("(n p) c -> p n c", p=128),
                    in_=ot3[:],
                )

    split_multi_waits(nc)
    return nc


_NC_CACHE = None


def get_nc():
    global _NC_CACHE
    if _NC_CACHE is None:
        _NC_CACHE = build_nc()
    return _NC_CACHE


def make_in_maps(H, U, W, b):
    H = np.ascontiguousarray(np.asarray(H, dtype=np.float32))
    U = np.ascontiguousarray(np.asarray(U, dtype=np.float32))
    W = np.ascontiguousarray(np.asarray(W, dtype=np.float32))
    b = np.ascontiguousarray(np.asarray(b, dtype=np.float32))
    return [
        {
            "H": H[i * BL : (i + 1) * BL],
            "U": U[i * BL : (i + 1) * BL],
            "W": W,
            "b": b,
        }
        for i in range(NCORES)
    ]


def assemble(results, H):
    """Unshard: stitch the exact-f32 H echo segment with the device-computed
    bf16 segments [c2q | H*c2q | H*q2c], upcast to f32."""
    H = np.asarray(H, dtype=np.float32)
    full = np.empty((B, T, 4 * D), dtype=np.float32)
    full[:, :, 0:D] = H
    rest = np.concatenate(
        [np.asarray(results[i]["out"]) for i in range(NCORES)], axis=0
    )
    full[:, :, D:] = rest.astype(np.float32)
    return full


def kernel(H, U, W, b):
    nc = get_nc()
    in_maps = make_in_maps(H, U, W, b)
    res = run_bass_kernel_spmd(nc, in_maps, core_ids=list(range(NCORES)))
    return assemble(res.results, H)


# revision 10
# speedup vs baseline: 1.3424x; 1.3424x over previous
"""Trainium2 Bass kernel for the BiDAF-style attention-flow layer.

S[b,t,j] = H.w_h + U.w_u + (H*w_hu).U + bias
c2q      = softmax_j(S) @ U
q2c      = softmax_t(max_j S) @ H   (broadcast over t)
out      = concat([H, c2q, H*c2q, H*q2c], axis=-1)

Sharding: data-parallel over batch B=64 across 8 NeuronCores (8 batches per
core); W/b replicated; no collectives.

The device computes and stores only the three derived segments
[c2q | H*c2q | H*q2c] in bf16; the H echo segment is assembled on the host
from the (exact f32) input during unsharding, and the bf16 segments are
upcast to f32 on the host. The rel-err budget (2e-2) is ~10x above bf16
rounding noise (~1e-3).

Structure (per batch of the 8 per core; t-tiles of 128, processed in PAIRS):
 - H, U are cast-loaded f32->bf16 during DMA (SWDGE).
 - S is computed TRANSPOSED: stp[j, t] (row 64 = sH) via half-batch matmuls
   with batch-constant stationary weights [w_hu*U^T | w_h].
 - ONE exp activation per pair produces both E^T = exp(S_core^T + sU + b)
   and exp(sH) (bias row 64 = 0), into et_ext [65, 256].
 - et_ext transposes back to [128, 2, 65]; row-max r' = max_j E and
   Z = sum_j E come from paired DVE reduces; em = exp(sH) * r'.
 - c2q matmuls write a PAIRED PSUM tile [128, 2, 256]; normalization is one
   paired DVE mult with a stride-0 broadcast of 1/Z; seg1 = seg0 * H is one
   paired bf16 mult; seg2 = H * q2c on gpsimd.
 - Normalization is software-pipelined one pair behind production so no
   engine queue blocks on a cross-engine round trip.
"""

import numpy as np

import concourse.bass as bass
import concourse.mybir as mybir
import concourse.tile as tile
from concourse.bass_utils import run_bass_kernel_spmd
from concourse.masks import make_identity

B, T, J, D = 64, 1024, 64, 256
NCORES = 8
BL = B // NCORES  # batches per core
NT = T // 128     # t-tiles per batch
NP = NT // 2      # tile pairs per batch
F32 = mybir.dt.float32
BF16 = mybir.dt.bfloat16
AX = mybir.AxisListType.X
AF = mybir.ActivationFunctionType
MUL = mybir.AluOpType.mult
ADD = mybir.AluOpType.add
MAX = mybir.AluOpType.max


def split_multi_waits(nc, max_waits=1):
    """Walrus in this container rejects instructions with more than a couple
    of embedded sync waits. Hoist extras into standalone EventSemaphore
    instructions right before the offending instruction."""
    n = 0
    for fn in nc.m.functions:
        for bb in fn.blocks:
            new_insts = []
            for inst in bb.instructions:
                si = getattr(inst, "sync_info", None)
                if si is not None and si.on_wait and len(si.on_wait) > max_waits:
                    waits = list(si.on_wait)
                    for w in waits[:-max_waits]:
                        n += 1
                        ev = mybir.InstEventSemaphore(
                            name=f"I-wsplit-{n}", ins=[], outs=[]
                        )
                        ev.engine = inst.engine
                        ev.sync_info = mybir.SyncInfo(on_wait=[w], on_update=[])
                        new_insts.append(ev)
                    inst.sync_info = mybir.SyncInfo(
                        on_wait=waits[-max_waits:], on_update=list(si.on_update)
                    )
                new_insts.append(inst)
            bb.instructions[:] = new_insts
    return n


def bcast0(ap, n):
    """Append a stride-0 dim of size n to an AP (free-axis broadcast)."""
    return bass.AP(tensor=ap.tensor, offset=ap.offset, ap=list(ap.ap) + [[0, n]])


def build_nc():
    nc = bass.Bass()
    H = nc.declare_dram_parameter("H", [BL, T, D], F32, isOutput=False)
    U = nc.declare_dram_parameter("U", [BL, J, D], F32, isOutput=False)
    W = nc.declare_dram_parameter("W", [3 * D], F32, isOutput=False)
    b = nc.declare_dram_parameter("b", [1], F32, isOutput=False)
    out = nc.declare_dram_parameter("out", [BL, T, 3 * D], BF16, isOutput=True)

    with tile.TileContext(nc) as tc:
        with (
            tc.tile_pool(name="singles", bufs=1) as singles,
            tc.tile_pool(name="hpool", bufs=2) as hpool,
            tc.tile_pool(name="upool", bufs=2) as upool,
            tc.tile_pool(name="htap", bufs=2) as htpool,
            tc.tile_pool(name="outp", bufs=3) as outp,
            tc.tile_pool(name="batch", bufs=2) as bpool,
            tc.tile_pool(name="small", bufs=4) as small,
            # PSUM: 2 + 2 + 1 + 2 + 1 = 8 banks exactly
            tc.tile_pool(name="ps_h", bufs=2, space="PSUM") as ps_h,
            tc.tile_pool(name="ps_s", bufs=1, space="PSUM") as ps_s,
            tc.tile_pool(name="ps_e", bufs=1, space="PSUM") as ps_e,
            tc.tile_pool(name="ps_c", bufs=2, space="PSUM") as ps_c,
            tc.tile_pool(name="ps_q", bufs=1, space="PSUM") as ps_q,
        ):
            ident_bf = singles.tile([128, 128], BF16)
            make_identity(nc, ident_bf[:])
            ones_row_bf = singles.tile([1, 128], BF16)
            nc.vector.memset(ones_row_bf[:], 1.0)

            # w_u broadcast over 64 partitions (for the sU reduction)
            w_u_bc = singles.tile([J, D], F32)
            wsl = W[D : 2 * D]
            nc.sync.dma_start(
                out=w_u_bc[:],
                in_=bass.AP(tensor=wsl.tensor, offset=wsl.offset,
                            ap=[[0, J]] + list(wsl.ap)),
            )
            # b broadcast over 64 partitions
            b_col = singles.tile([J, 1], F32)
            bsl = b[0:1]
            nc.sync.dma_start(
                out=b_col[:],
                in_=bass.AP(tensor=bsl.tensor, offset=bsl.offset,
                            ap=[[0, J]] + list(bsl.ap)),
            )
            # w_hu and w_h as [128,2] column blocks
            whu_col = singles.tile([128, 2], F32)
            wh_col = singles.tile([128, 2], F32)
            for k in range(2):
                nc.sync.dma_start(
                    out=whu_col[:, k : k + 1],
                    in_=W[2 * D + 128 * k : 2 * D + 128 * (k + 1)].rearrange(
                        "(p o) -> p o", o=1
                    ),
                )
                nc.sync.dma_start(
                    out=wh_col[:, k : k + 1],
                    in_=W[128 * k : 128 * (k + 1)].rearrange("(p o) -> p o", o=1),
                )

            def load_batch(bi):
                # H cast-loaded f32 -> bf16 during DMA (SWDGE), with a ones
                # column in front of each tile block (for the q2czt matmul).
                hb = hpool.tile([128, NT, D + 1], BF16, tag="hb")
                nc.gpsimd.memset(hb[:, :, 0:1], 1.0)
                nc.gpsimd.dma_start(
                    out=hb[:, :, 1 : D + 1],
                    in_=H[bi].rearrange("(n p) d -> p n d", p=128),
                )
                ub = upool.tile([J, D], BF16, tag="ub")
                nc.gpsimd.dma_start(out=ub[:], in_=U[bi])
                return hb, ub

            cur = load_batch(0)

            for bi in range(BL):
                hb, ub = cur
                if bi + 1 < BL:
                    cur = load_batch(bi + 1)

                # ---- per-batch U prep ------------------------------------
                su_scr = bpool.tile([J, D], F32, tag="suscr")
                su_raw = bpool.tile([J, 1], F32, tag="suraw")
                nc.gpsimd.tensor_mul(su_scr[:], ub[:], w_u_bc[:])
                nc.vector.reduce_sum(su_raw[:], su_scr[:], axis=AX, op=ADD)
                # su_ext rows 0:64 = sU + b, row 64 = 0 (so the paired exp
                # also yields exp(sH) in row 64)
                su_ext = bpool.tile([J + 1, 1], F32, tag="suext")
                nc.vector.tensor_tensor(
                    out=su_ext[0:J, :], in0=su_raw[:], in1=b_col[:], op=ADD
                )
                nc.vector.memset(su_ext[J : J + 1, :], 0.0)

                # U^T chunks -> stationary weights [w_hu*U^T | w_h]
                utp = ps_e.tile([128, 2, J], BF16, tag="et")
                nc.tensor.transpose(utp[:, 0, :], ub[:, 0:128],
                                    ident_bf[0:J, 0:J])
                nc.tensor.transpose(utp[:, 1, :], ub[:, 128:256],
                                    ident_bf[0:J, 0:J])
                rhs_w = bpool.tile([128, 2, J + 1], BF16, tag="rhsw")
                for c in range(2):
                    nc.vector.tensor_scalar_mul(
                        rhs_w[:, c, 0:J], utp[:, c, :], whu_col[:, c : c + 1]
                    )
                    nc.scalar.copy(rhs_w[:, c, J : J + 1], wh_col[:, c : c + 1])

                # ---- H^T (pairs), S^T (halves) ---------------------------
                ht_all = htpool.tile([128, 2, T], BF16, tag="hta")
                stps = []

                def do_pair_transpose(p):
                    htp = ps_h.tile([128, 2, 2, 128], BF16, tag="ht")
                    for k in range(2):
                        ti = 2 * p + k
                        nc.tensor.transpose(htp[:, 0, k, :],
                                            hb[:, ti, 1:129], ident_bf[:])
                        nc.tensor.transpose(htp[:, 1, k, :],
                                            hb[:, ti, 129:257], ident_bf[:])
                    nc.scalar.copy(
                        ht_all[:, :, 256 * p : 256 * (p + 1)].rearrange(
                            "p c (k q) -> p c k q", k=2
                        ),
                        htp[:],
                    )

                def do_half_s(h):
                    stp = ps_s.tile([J + 1, T // 2], F32, tag=f"s{h}")
                    for c in range(2):
                        nc.tensor.matmul(
                            stp[:], rhs_w[:, c, :],
                            ht_all[:, c, 512 * h : 512 * (h + 1)],
                            start=(c == 0), stop=(c == 1),
                        )
                    stps.append(stp)

                do_pair_transpose(0)
                do_pair_transpose(1)
                do_half_s(0)
                do_pair_transpose(2)
                do_pair_transpose(3)
                do_half_s(1)

                # ---- paired softmax + c2q pipeline -----------------------
                ot3 = outp.tile([128, NT, 3 * D], BF16, tag="ot")
                q2czt = ps_q.tile([1, D + 1], F32, tag="qz")

                pend = [None, None]  # lagged stages: [normalize(p-1), q2czt(p-1)]

                def stage_norm(st):
                    cq3, p, zs = st
                    zinv = small.tile([128, 2], F32, tag="zinv")
                    nc.vector.reciprocal(zinv[:], zs[:])
                    nc.vector.tensor_tensor(
                        out=ot3[:, 2 * p : 2 * p + 2, 0:D],
                        in0=cq3[:], in1=bcast0(zinv[:], D), op=MUL,
                    )
                    nc.vector.tensor_tensor(
                        out=ot3[:, 2 * p : 2 * p + 2, D : 2 * D],
                        in0=ot3[:, 2 * p : 2 * p + 2, 0:D],
                        in1=hb[:, 2 * p : 2 * p + 2, 1 : D + 1], op=MUL,
                    )

                def stage_q2czt(st):
                    em, p = st
                    for k in range(2):
                        ti = 2 * p + k
                        nc.tensor.matmul(
                            q2czt[0:1, :], em[:, k : k + 1], hb[:, ti, :],
                            start=(ti == 0), stop=(ti == NT - 1),
                            skip_group_check=True,
                        )

                for p in range(NP):
                    h, q = divmod(p, 2)
                    scols = slice(256 * q, 256 * (q + 1))
                    stp = stps[h]
                    # E^T-pair (+ exp(sH) in row 64), one activation
                    et_ext = small.tile([J + 1, 256], BF16, tag="etx")
                    nc.scalar.activation(et_ext[:], stp[:, scols],
                                         AF.Exp, bias=su_ext[:], scale=1.0)
                    # transpose back: etT[:, k, 0:64] = E, [:, k, 64] = exp(sH)
                    # inner dim padded to 66 so the k=1 block is 4B-aligned
                    etT = ps_e.tile([128, 2, J + 2], BF16, tag="et")
                    for k in range(2):
                        nc.tensor.transpose(
                            etT[:, k, 0 : J + 1],
                            et_ext[:, 128 * k : 128 * (k + 1)],
                            ident_bf[0 : J + 1, 0 : J + 1],
                        )
                    # c2q (unnormalized) into the paired PSUM tile
                    cq3 = ps_c.tile([128, 2, D], F32, tag="cq")
                    for k in range(2):
                        nc.tensor.matmul(
                            cq3[:, k, :], et_ext[0:J, 128 * k : 128 * (k + 1)],
                            ub[:], start=True, stop=True,
                        )
                    if pend[1] is not None:
                        stage_q2czt(pend[1])
                    # r' = max_j E ; Z = sum_j E ; em = exp(sH) * r'
                    r = small.tile([128, 2], BF16, tag="r")
                    nc.vector.reduce_max(r[:], etT[:, :, 0:J], axis=AX, op=MAX)
                    zs = small.tile([128, 2], F32, tag="zs")
                    nc.vector.reduce_sum(zs[:], etT[:, :, 0:J], axis=AX, op=ADD)
                    em = small.tile([128, 2], BF16, tag="em")
                    nc.vector.tensor_tensor(
                        out=em[:], in0=etT[:, :, J], in1=r[:], op=MUL
                    )
                    if pend[0] is not None:
                        stage_norm(pend[0])
                    pend = [(cq3, p, zs), (em, p)]

                stage_q2czt(pend[1])
                stage_norm(pend[0])

                # ---- q2c broadcast + pass 2 ------------------------------
                ztinv = bpool.tile([1, 1], F32, tag="ztinv")
                nc.vector.reciprocal(ztinv[:], q2czt[0:1, 0:1])
                q2c_row = bpool.tile([1, D], BF16, tag="q2crow")
                nc.vector.tensor_scalar_mul(q2c_row[:], q2czt[0:1, 1 : D + 1],
                                            ztinv[:])
                q2cbp = ps_q.tile([128, D], F32, tag="qz")
                nc.tensor.matmul(q2cbp[:], ones_row_bf[:], q2c_row[:],
                                 start=True, stop=True)
                q2cb = bpool.tile([128, D], BF16, tag="q2cb")
                nc.scalar.copy(q2cb[:], q2cbp[:])

                for ti in range(NT):
                    eng = nc.gpsimd if ti % 2 == 0 else nc.vector
                    eng.tensor_mul(
                        ot3[:, ti, 2 * D : 3 * D], hb[:, ti, 1 : D + 1],
                        q2cb[:],
                    )
                # one batched store of all three segments
                nc.scalar.dma_start(
                    out=out[bi].rearrange("(n p) c -> p n c", p=128),
                    in_=ot3[:],
                )

    split_multi_waits(nc)
    return nc


_NC_CACHE = None


def get_nc():
    global _NC_CACHE
    if _NC_CACHE is None:
        _NC_CACHE = build_nc()
    return _NC_CACHE


def make_in_maps(H, U, W, b):
    H = np.ascontiguousarray(np.asarray(H, dtype=np.float32))
    U = np.ascontiguousarray(np.asarray(U, dtype=np.float32))
    W = np.ascontiguousarray(np.asarray(W, dtype=np.float32))
    b = np.ascontiguousarray(np.asarray(b, dtype=np.float32))
    return [
        {
            "H": H[i * BL : (i + 1) * BL],
            "U": U[i * BL : (i + 1) * BL],
            "W": W,
            "b": b,
        }
        for i in range(NCORES)
    ]


def assemble(results, H):
    """Unshard: stitch the exact-f32 H echo segment with the device-computed
    bf16 segments [c2q | H*c2q | H*q2c], upcast to f32."""
    H = np.asarray(H, dtype=np.float32)
    full = np.empty((B, T, 4 * D), dtype=np.float32)
    full[:, :, 0:D] = H
    rest = np.concatenate(
        [np.asarray(results[i]["out"]) for i in range(NCORES)], axis=0
    )
    full[:, :, D:] = rest.astype(np.float32)
    return full


def kernel(H, U, W, b):
    nc = get_nc()
    in_maps = make_in_maps(H, U, W, b)
    res = run_bass_kernel_spmd(nc, in_maps, core_ids=list(range(NCORES)))
    return assemble(res.results, H)
